# revision 14
# baseline (speedup 1.0000x reference)
"""Trainium2 Bass kernel for nn_CrossAttention (B=4, C=256, H=W=64).

Sharding: 8 cores = (batch b, query-half h). Each core computes, for its
batch and its half of the query rows i (IH=2048):
  q = Wq x_i + bq        [32, 2048] stored 4x row-replicated as q4 [128, 2048]
  k = Wk x_f             [32, 4096] stored 4x row-replicated as k4 [128, 4096]
                         (bk dropped: constant-in-j shift is softmax-invariant)
  vT = (Wv x_f)^T        [4096, 256] bf16  (bv folded into bc_eff on host)
  S^T[j, i] = k_j . q_i  (transposed layout; row-replication lets two K=32
                          score matmuls run concurrently in PE row strips)
  E = exp(S^T) bf16      (no max subtraction: |S| <~ 30, exp safe in f32)
  att_raw[c, i] = sum_j vT[j, c] E[j, i]   (UN-normalized)
  r[i] = sum_j E[j, i]
  out[i] = (1/r) sum_c |r~ (Wcx x_i + bc) + Wca att_raw|   (r~ = bf16(r);
           the softmax 1/r normalization is algebraically deferred through
           the linear combine and pulled out of the abs-sum)

Engine budget per query block (~17.4us of PE work): exp on ACT ~16us, the
r partial sums as one DVE pair-add per group with accumulation on the DMA
engines (two alternating chains; DVE was the bottleneck when it held the
whole chain), partition fold of r as a single f32 ones-matmul into the 8th
PSUM bank, recip via reciprocal_approx_fast. The r tail for block ib is
EMITTED inside block ib+1's group loop so the in-order PE queue never
stalls on it (PE idle >3.4us re-throttles the clock to 1.2GHz via HAM).
PSUM: 4 score staging + 3 attended + 1 r = 8 banks.
"""

import numpy as np
import ml_dtypes

import concourse.bass as bass
import concourse.bacc as bacc
import concourse.tile as tile
import concourse.mybir as mybir
from concourse.bass_utils import run_bass_kernel_spmd

B, C, HH, WW = 4, 256, 64, 64
N = HH * WW          # 4096
CQK = 32
IH = N // 2          # 2048 query rows per core
NCORES = 8
NJC = N // 128       # 32 key-dim 128-chunks
NG = NJC // 2        # 16 groups of 2 key-chunks

F32 = mybir.dt.float32
F32R = mybir.dt.float32r
BF16 = mybir.dt.bfloat16
AF = mybir.ActivationFunctionType
ALU = mybir.AluOpType


def build_program(nc, tc):
    # ---- DRAM I/O ------------------------------------------------------
    dram = {}
    for name, shape, dt in [
        ("x1f", [2, 128, N], BF16), ("x2f", [2, 128, N], BF16),
        ("x1i", [2, 128, IH], BF16), ("x2i", [2, 128, IH], BF16),
        ("wqt", [2, 128, 128], BF16), ("wkt", [2, 128, 128], BF16),
        ("wvt", [2, 128, C], BF16),
        ("wctx", [2, 128, C], BF16), ("wcta", [2, 128, C], BF16),
        ("bq", [128, 1], F32), ("bce", [1, 2, 128], BF16),
        ("onesc", [128, 1], F32R),
    ]:
        dram[name] = nc.dram_tensor(name, shape, dt, kind="ExternalInput").ap()
    out_d = nc.dram_tensor("out", [2, IH], F32, kind="ExternalOutput").ap()

    import contextlib
    with contextlib.ExitStack() as ctx:
        persist = ctx.enter_context(tc.tile_pool(name="persist", bufs=1))

        wq_sb = persist.tile([128, 2, 128], BF16, tag="wq")
        wk_sb = persist.tile([128, 2, 128], BF16, tag="wk")
        wv_sb = persist.tile([128, 2, C], BF16, tag="wv")
        wcx_sb = persist.tile([128, 2, C], BF16, tag="wcx")
        wca_sb = persist.tile([128, 2, C], BF16, tag="wca")
        bq_sb = persist.tile([128, 1], F32, tag="bq")
        bce_sb = persist.tile([1, 2, 128], BF16, tag="bce")
        ones_bf = persist.tile([128, 1], BF16, tag="ones")
        ones_f = persist.tile([128, 1], F32R, tag="onesf")
        x1i_sb = [persist.tile([128, IH], BF16, tag=f"x1i{kc}",
                               name=f"x1i{kc}") for kc in range(2)]

        # DMA issue order = need order: q-projection inputs first
        nc.sync.dma_start(out=bq_sb, in_=dram["bq"])
        for kc in range(2):
            nc.sync.dma_start(out=wq_sb[:, kc, :], in_=dram["wqt"][kc])
        for kc in range(2):
            nc.sync.dma_start(out=x1i_sb[kc], in_=dram["x1i"][kc])
        for w, t in [("wkt", wk_sb), ("wvt", wv_sb)]:
            for kc in range(2):
                nc.sync.dma_start(out=t[:, kc, :], in_=dram[w][kc])
        nc.vector.memset(ones_bf, 1.0)
        nc.sync.dma_start(out=ones_f, in_=dram["onesc"])

        # projection outputs; k4/vT split in j-halves for earlier consumption
        q4_sb = [persist.tile([128, IH], BF16, tag=f"q{i}", name=f"q{i}")
                 for i in range(2)]
        k4_sb = [[persist.tile([128, N // 2], BF16, tag=f"k{i}{h}",
                               name=f"k{i}{h}") for h in range(2)]
                 for i in range(2)]
        vT_sb = [[persist.tile([128, (NJC // 2) * C], BF16, tag=f"vt{i}{h}",
                               name=f"vt{i}{h}") for h in range(2)]
                 for i in range(2)]
        att_sb = [[persist.tile([128, IH], BF16, tag=f"att{br}{c2}",
                                name=f"att{br}{c2}") for c2 in range(2)]
                  for br in range(2)]
        # per (br, ib) softmax-denominator products, consumed by phase 2b
        rb_sb = [[persist.tile([128, 512], BF16, tag=f"rb{br}{ib}",
                               name=f"rb{br}{ib}") for ib in range(4)]
                 for br in range(2)]
        rlb_sb = [[persist.tile([1, 512], BF16, tag=f"rlb{br}{ib}",
                                name=f"rlb{br}{ib}") for ib in range(4)]
                  for br in range(2)]
        rr_sb = [[persist.tile([1, 512], F32, tag=f"rr{br}{ib}",
                               name=f"rr{br}{ib}") for ib in range(4)]
                 for br in range(2)]

        # ---- phase 1: projections -------------------------------------
        with tc.tile_pool(name="proj_sb", bufs=2) as proj_sb, \
             tc.tile_pool(name="ps_kq", bufs=3, space="PSUM") as ps_kq, \
             tc.tile_pool(name="ps_vt", bufs=2, space="PSUM") as ps_vt:

            # q4 projections (from islice inputs; bq folded via ACT bias)
            for xi in range(2):
                if xi == 0:
                    xi_sb = x1i_sb
                else:
                    xi_sb = [proj_sb.tile([128, IH], BF16, tag="x2i",
                                          name="x2i") for _ in range(2)]
                    for kc in range(2):
                        nc.sync.dma_start(out=xi_sb[kc], in_=dram["x2i"][kc])
                for ib in range(4):
                    sl = bass.ts(ib, 512)
                    qp = ps_kq.tile([128, 512], F32, tag="kq", name="qp")
                    for kc in range(2):
                        nc.tensor.matmul(qp, wq_sb[:, kc, :], xi_sb[kc][:, sl],
                                         start=(kc == 0), stop=(kc == 1))
                    nc.scalar.activation(q4_sb[xi][:, sl], qp, AF.Identity,
                                         bias=bq_sb)

            # k4 and vT projections, x2 first (branch 0 needs vT2)
            for xi, xf_name in [(1, "x2f"), (0, "x1f")]:
                for jh in range(2):
                    xf_t = proj_sb.tile([128, 2, IH], BF16, tag="xf",
                                        name="xf")
                    for kc in range(2):
                        nc.sync.dma_start(
                            out=xf_t[:, kc, :],
                            in_=dram[xf_name][kc][:, jh * IH:(jh + 1) * IH])
                    for jb in range(4):
                        sl = bass.ts(jb, 512)
                        kp = ps_kq.tile([128, 512], F32, tag="kq", name="kp")
                        for kc in range(2):
                            nc.tensor.matmul(kp, wk_sb[:, kc, :],
                                             xf_t[:, kc, sl],
                                             start=(kc == 0), stop=(kc == 1))
                        nc.scalar.activation(k4_sb[xi][jh][:, sl], kp, AF.Copy)
                    for g in range(4):
                        vtp = ps_vt.tile([128, 4, C], F32, tag="vt",
                                         name="vtp")
                        for s in range(4):
                            jsub = g * 4 + s
                            for kc in range(2):
                                nc.tensor.matmul(
                                    vtp[:, s, :],
                                    xf_t[:, kc, bass.ts(jsub, 128)],
                                    wv_sb[:, kc, :],
                                    start=(kc == 0), stop=(kc == 1))
                        nc.vector.tensor_copy(
                            vT_sb[xi][jh][:, bass.ds(g * 4 * C, 4 * C)],
                            vtp.rearrange("p a c -> p (a c)"))
            nc.sync.dma_start(out=bce_sb, in_=dram["bce"])
            for kc in range(2):
                nc.sync.dma_start(out=wcx_sb[:, kc, :], in_=dram["wctx"][kc])
                nc.sync.dma_start(out=wca_sb[:, kc, :], in_=dram["wcta"][kc])

        # r-fold PSUM bank and r-accumulation SBUF outlive 2a (the final
        # block's fold is emitted from inside phase 2b)
        ps_rp = ctx.enter_context(
            tc.tile_pool(name="ps_rp", bufs=1, space="PSUM"))
        racc_pool = ctx.enter_context(tc.tile_pool(name="racc_sb", bufs=1))

        # ---- phase 2a: attention (att_sb <- raw attended; r products) --
        with tc.tile_pool(name="attn_sb", bufs=1) as attn_sb, \
             tc.tile_pool(name="ps_att", bufs=1, space="PSUM") as ps_att, \
             tc.tile_pool(name="ps_st", bufs=1, space="PSUM") as ps_st:

            def r_tail(br, ib, racc_a, racc_b):
                """Fold r partials + derive products. Emitted deferred (from
                inside the NEXT block's group loop) so no PE instruction
                ever waits on the DMA-accumulate chains."""
                rp = ps_rp.tile([1, 512], F32, tag="rp", bufs=1, name="rp")
                nc.tensor.matmul(rp, ones_f, racc_a, start=True, stop=False)
                nc.tensor.matmul(rp, ones_f, racc_b, start=False, stop=True)
                # 1/r (f32, ~18-bit) straight from PSUM
                nc.vector.reciprocal_approx_fast(rr_sb[br][ib], rp)
                # r line -> bf16 (ACT; DVE is the tighter engine), then
                # partition-broadcast for the 2b x1-prescale
                nc.scalar.activation(rlb_sb[br][ib], rp, AF.Copy)
                nc.gpsimd.partition_broadcast(rb_sb[br][ib], rlb_sb[br][ib])

            pending = None
            for br in range(2):
                q4, k4, vT = q4_sb[br], k4_sb[br], vT_sb[1 - br]
                for ib in range(4):
                    isl = bass.ts(ib, 512)
                    attp = [ps_att.tile([128, 512], F32, tag="attp",
                                        bufs=3, name=f"attp{c2}")
                            for c2 in range(2)]
                    racc_v = None     # DVE ping-pong chain (even g)
                    racc_d = racc_pool.tile([128, 512], F32R, tag="raccd",
                                            bufs=2, name="racc_d")
                    for g in range(NG):
                        jcs = (2 * g, 2 * g + 1)
                        jh = g // (NG // 2)
                        jloc = [jc - jh * (NJC // 2) for jc in jcs]
                        stp = ps_st.tile([128, 2, 512], F32, tag="stp",
                                         bufs=2, name="stp")
                        # score matmuls: 2 row strips run concurrently
                        for t in range(2):
                            nc.tensor.matmul(
                                stp[:, t, :],
                                k4[jh][32 * t:32 * (t + 1),
                                       bass.ts(jloc[t], 128)],
                                q4[32 * t:32 * (t + 1), isl],
                                start=True, stop=True,
                                tile_position=(32 * t, 0))
                        est = attn_sb.tile([128, 2, 512], BF16,
                                           tag="est", bufs=8, name="est")
                        nc.scalar.activation(
                            est.rearrange("p a n -> p (a n)"),
                            stp.rearrange("p a n -> p (a n)"), AF.Exp)
                        # attended (un-normalized)
                        for t in range(2):
                            for c2 in range(2):
                                nc.tensor.matmul(
                                    attp[c2],
                                    vT[jh][:, bass.ds(jloc[t] * C
                                                      + c2 * 128, 128)],
                                    est[:, t, :],
                                    start=(g == 0 and t == 0),
                                    stop=(g == NG - 1 and t == 1))
                        # r partials: pair-sum the two strips, then
                        # accumulate -- even groups chain on the DVE
                        # (ping-pong, no in-place op), odd groups on one
                        # DMA-accumulate chain; splitting keeps both the
                        # DVE and the gpsimd trigger queue under the PE
                        # group time
                        if g % 2 == 0:
                            rtmp = racc_pool.tile([128, 512], BF16,
                                                  tag="rtmpb", bufs=2,
                                                  name="rtmp_b")
                            nc.vector.tensor_tensor(rtmp, est[:, 0, :],
                                                    est[:, 1, :], ALU.add)
                            rv = racc_pool.tile([128, 512], F32R,
                                                tag="raccv", bufs=2,
                                                name="racc_v")
                            if racc_v is None:
                                nc.vector.tensor_copy(rv, rtmp)
                            else:
                                nc.vector.tensor_tensor(rv, racc_v, rtmp,
                                                        ALU.add)
                            racc_v = rv
                        elif g == 1:
                            nc.vector.tensor_tensor(racc_d, est[:, 0, :],
                                                    est[:, 1, :], ALU.add)
                        else:
                            rtmp = racc_pool.tile([128, 512], F32R,
                                                  tag="rtmp", bufs=3,
                                                  name="rtmp")
                            nc.vector.tensor_tensor(rtmp, est[:, 0, :],
                                                    est[:, 1, :], ALU.add)
                            nc.gpsimd.dma_start(out=racc_d, in_=rtmp,
                                                accum_op=ALU.add)
                        if g == 8 and pending is not None:
                            r_tail(*pending)
                            pending = None
                    # release attp immediately (cast only -- r not needed)
                    for c2 in range(2):
                        nc.vector.tensor_copy(att_sb[br][c2][:, isl],
                                              attp[c2])
                    pending = (br, ib, racc_v, racc_d)
            last_pending = pending

        # ---- phase 2b: combines, back-to-back on the PE ----------------
        with tc.tile_pool(name="cmb_sb", bufs=1) as cmb_sb, \
             tc.tile_pool(name="ps_cmb", bufs=1, space="PSUM") as ps_cmb:

            for br in range(2):
                for ib in range(4):
                    isl = bass.ts(ib, 512)
                    x1r = cmb_sb.tile([128, 2, 512], BF16, tag="x1r",
                                      bufs=2, name="x1r")
                    for kc in range(2):
                        nc.vector.tensor_tensor(x1r[:, kc, :],
                                                x1i_sb[kc][:, isl],
                                                rb_sb[br][ib], ALU.mult)
                    absb = []
                    for c2 in range(2):
                        cp = ps_cmb.tile([128, 512], F32, tag="cp",
                                         bufs=3, name="cp")
                        for kc in range(2):
                            nc.tensor.matmul(
                                cp, wcx_sb[:, kc, bass.ts(c2, 128)],
                                x1r[:, kc, :],
                                start=(kc == 0), stop=False)
                        nc.tensor.matmul(cp, bce_sb[:, c2, :],
                                         rlb_sb[br][ib],
                                         start=False, stop=False)
                        for kc in range(2):
                            nc.tensor.matmul(
                                cp, wca_sb[:, kc, bass.ts(c2, 128)],
                                att_sb[br][kc][:, isl],
                                start=False, stop=(kc == 1))
                        ab = cmb_sb.tile([128, 512], BF16, tag="absb",
                                         bufs=4, name="absb")
                        nc.scalar.activation(ab, cp, AF.Abs)
                        absb.append(ab)
                        if br == 0 and ib == 2 and c2 == 1:
                            # fold the final block's r chain here, long
                            # after its DMA accumulates have drained
                            r_tail(*last_pending)
                    outp = ps_cmb.tile([1, 512], F32, tag="outp", bufs=2,
                                       name="outp")
                    for c2 in range(2):
                        nc.tensor.matmul(outp, ones_bf, absb[c2],
                                         start=(c2 == 0), stop=(c2 == 1))
                    osb = cmb_sb.tile([1, 512], F32, tag="osb", bufs=2,
                                      name="osb")
                    nc.vector.tensor_tensor(osb, outp, rr_sb[br][ib],
                                            ALU.mult)
                    nc.sync.dma_start(out=out_d[br:br + 1, isl], in_=osb)


_NC_CACHE = {}


def _get_nc():
    if "nc" not in _NC_CACHE:
        nc = bacc.Bacc("TRN2", debug=False, enable_asserts=False,
                       target_bir_lowering=False, enable_partition_id=False)
        with tile.TileContext(nc) as tc:
            build_program(nc, tc)
        nc.compile()
        _NC_CACHE["nc"] = nc
    return _NC_CACHE["nc"]


def host_inputs(x1, x2, Wq, bq, Wk, bk, Wv, bv, Wc, bc):
    """Build the 8 per-core input maps (host-side sharding/layout only)."""
    f = np.float32
    bf = ml_dtypes.bfloat16
    x1 = np.asarray(x1, f); x2 = np.asarray(x2, f)
    Wq = np.asarray(Wq, f); bq = np.asarray(bq, f)
    Wk = np.asarray(Wk, f)
    Wv = np.asarray(Wv, f); bv = np.asarray(bv, f)
    Wc = np.asarray(Wc, f); bc = np.asarray(bc, f)

    # 4x row-replicated q/k projection weights -> q4/k4 [128, n] layouts
    Wq4 = np.tile(Wq, (4, 1))            # [128, 256]
    Wk4 = np.tile(Wk, (4, 1))
    wqt = np.ascontiguousarray(Wq4.T.reshape(2, 128, 128)).astype(bf)
    wkt = np.ascontiguousarray(Wk4.T.reshape(2, 128, 128)).astype(bf)
    bq4 = np.tile(bq, 4).reshape(128, 1).copy()
    wvt = np.ascontiguousarray(Wv.T.reshape(2, 128, C)).astype(bf)
    WcT = np.ascontiguousarray(Wc.T)     # [512, 256]
    wctx = WcT[:C].reshape(2, 128, C).astype(bf)
    wcta = WcT[C:].reshape(2, 128, C).astype(bf)
    bce = (bc + Wc[:, C:] @ bv).reshape(1, 2, 128).astype(bf)

    in_maps = []
    for core in range(NCORES):
        b, h = divmod(core, 2)
        x1f = x1[b].reshape(C, N).reshape(2, 128, N)
        x2f = x2[b].reshape(C, N).reshape(2, 128, N)
        in_maps.append({
            "x1f": np.ascontiguousarray(x1f).astype(bf),
            "x2f": np.ascontiguousarray(x2f).astype(bf),
            "x1i": np.ascontiguousarray(
                x1f[:, :, h * IH:(h + 1) * IH]).astype(bf),
            "x2i": np.ascontiguousarray(
                x2f[:, :, h * IH:(h + 1) * IH]).astype(bf),
            "wqt": wqt, "wkt": wkt, "wvt": wvt,
            "wctx": wctx, "wcta": wcta,
            "bq": bq4, "bce": bce,
            "onesc": np.ones((128, 1), f),
        })
    return in_maps


def assemble(results):
    """results: list of 8 dicts with 'out' [2, IH] -> (out1, out2) full."""
    outs = []
    for row in range(2):
        full = np.empty((B, 1, HH, WW), np.float32)
        for b in range(B):
            half0 = results[2 * b]["out"][row]
            half1 = results[2 * b + 1]["out"][row]
            full[b, 0] = np.concatenate([half0, half1]).reshape(HH, WW)
        outs.append(full)
    return outs[0], outs[1]


def kernel(x1, x2, Wq, bq, Wk, bk, Wv, bv, Wc, bc):
    in_maps = host_inputs(x1, x2, Wq, bq, Wk, bk, Wv, bv, Wc, bc)
    nc = _get_nc()
    res = run_bass_kernel_spmd(nc, in_maps, core_ids=list(range(NCORES)))
    return assemble(res.results)


# revision 15
# speedup vs baseline: 1.3190x; 1.3190x over previous
"""Trainium2 Bass kernel for nn_CrossAttention (B=4, C=256, H=W=64).

Sharding: 8 cores = (batch b, query-half h). Each core computes, for its
batch and its half of the query rows i (IH=2048):
  q = Wq x_i + bq        [32, 2048] stored 4x row-replicated as q4 [128, 2048]
  k = Wk x_f             [32, 4096] stored 4x row-replicated as k4 [128, 4096]
                         (bk dropped: constant-in-j shift is softmax-invariant)
  vT = (Wv x_f)^T        [4096, 256] bf16  (bv folded into bc_eff on host)
  S^T[j, i] = k_j . q_i  (transposed layout; row-replication lets two K=32
                          score matmuls run concurrently in PE row strips)
  E = exp(S^T) bf16      (no max subtraction: |S| <~ 30, exp safe in f32)
  att_raw[c, i] = sum_j vT[j, c] E[j, i]   (UN-normalized)
  r[i] = sum_j E[j, i]
  out[i] = (1/r) sum_c |r~ (Wcx x_i + bc) + Wca att_raw|   (r~ = bf16(r);
           the softmax 1/r normalization is algebraically deferred through
           the linear combine and pulled out of the abs-sum)

Engine budget per query block (~17.4us of PE work): exp on ACT ~16us, the
r partial sums as one DVE pair-add per group with accumulation on the DMA
engines (two alternating chains; DVE was the bottleneck when it held the
whole chain), partition fold of r as a single f32 ones-matmul into the 8th
PSUM bank, recip via reciprocal_approx_fast. The r tail for block ib is
EMITTED inside block ib+1's group loop so the in-order PE queue never
stalls on it (PE idle >3.4us re-throttles the clock to 1.2GHz via HAM).
PSUM: 4 score staging + 3 attended + 1 r = 8 banks.
"""

import numpy as np
import ml_dtypes

import concourse.bass as bass
import concourse.bacc as bacc
import concourse.tile as tile
import concourse.mybir as mybir
from concourse.bass_utils import run_bass_kernel_spmd

B, C, HH, WW = 4, 256, 64, 64
N = HH * WW          # 4096
CQK = 32
IH = N // 2          # 2048 query rows per core
NCORES = 8
NJC = N // 128       # 32 key-dim 128-chunks
NG = NJC // 2        # 16 groups of 2 key-chunks

F32 = mybir.dt.float32
F32R = mybir.dt.float32r
BF16 = mybir.dt.bfloat16
AF = mybir.ActivationFunctionType
ALU = mybir.AluOpType


def build_program(nc, tc):
    # ---- DRAM I/O ------------------------------------------------------
    dram = {}
    for name, shape, dt in [
        ("x1f", [2, 128, N], BF16), ("x2f", [2, 128, N], BF16),
        ("x1i", [2, 128, IH], BF16), ("x2i", [2, 128, IH], BF16),
        ("wqt", [2, 128, 128], BF16), ("wkt", [2, 128, 128], BF16),
        ("wvt", [2, 128, C], BF16),
        ("wctx", [2, 128, C], BF16), ("wcta", [2, 128, C], BF16),
        ("bq", [128, 1], F32), ("bce", [1, 2, 128], BF16),
        ("onesc", [128, 1], F32R),
    ]:
        dram[name] = nc.dram_tensor(name, shape, dt, kind="ExternalInput").ap()
    out_d = nc.dram_tensor("out", [2, IH], F32, kind="ExternalOutput").ap()

    import contextlib
    with contextlib.ExitStack() as ctx:
        persist = ctx.enter_context(tc.tile_pool(name="persist", bufs=1))

        wq_sb = persist.tile([128, 2, 128], BF16, tag="wq")
        wk_sb = persist.tile([128, 2, 128], BF16, tag="wk")
        wv_sb = persist.tile([128, 2, C], BF16, tag="wv")
        wcx_sb = persist.tile([128, 2, C], BF16, tag="wcx")
        wca_sb = persist.tile([128, 2, C], BF16, tag="wca")
        bq_sb = persist.tile([128, 1], F32, tag="bq")
        bce_sb = persist.tile([1, 2, 128], BF16, tag="bce")
        ones_bf = persist.tile([128, 1], BF16, tag="ones")
        ones_f = persist.tile([128, 1], F32R, tag="onesf")
        x1i_sb = [persist.tile([128, IH], BF16, tag=f"x1i{kc}",
                               name=f"x1i{kc}") for kc in range(2)]

        # DMA issue order = need order: q-projection inputs first
        nc.sync.dma_start(out=bq_sb, in_=dram["bq"])
        for kc in range(2):
            nc.sync.dma_start(out=wq_sb[:, kc, :], in_=dram["wqt"][kc])
        for kc in range(2):
            nc.sync.dma_start(out=x1i_sb[kc], in_=dram["x1i"][kc])
        for w, t in [("wkt", wk_sb), ("wvt", wv_sb)]:
            for kc in range(2):
                nc.sync.dma_start(out=t[:, kc, :], in_=dram[w][kc])
        nc.vector.memset(ones_bf, 1.0)
        nc.sync.dma_start(out=ones_f, in_=dram["onesc"])

        # projection outputs; k4/vT split in j-halves for earlier consumption
        q4_sb = [persist.tile([128, IH], BF16, tag=f"q{i}", name=f"q{i}")
                 for i in range(2)]
        k4_sb = [[persist.tile([128, N // 2], BF16, tag=f"k{i}{h}",
                               name=f"k{i}{h}") for h in range(2)]
                 for i in range(2)]
        vT_sb = [[persist.tile([128, (NJC // 2) * C], BF16, tag=f"vt{i}{h}",
                               name=f"vt{i}{h}") for h in range(2)]
                 for i in range(2)]
        att_sb = [[persist.tile([128, IH], BF16, tag=f"att{br}{c2}",
                                name=f"att{br}{c2}") for c2 in range(2)]
                  for br in range(2)]
        # per (br, ib) softmax-denominator products, consumed by phase 2b
        rb_sb = [[persist.tile([128, 512], BF16, tag=f"rb{br}{ib}",
                               name=f"rb{br}{ib}") for ib in range(4)]
                 for br in range(2)]
        rlb_sb = [[persist.tile([1, 512], BF16, tag=f"rlb{br}{ib}",
                                name=f"rlb{br}{ib}") for ib in range(4)]
                  for br in range(2)]
        rr_sb = [[persist.tile([1, 512], F32, tag=f"rr{br}{ib}",
                               name=f"rr{br}{ib}") for ib in range(4)]
                 for br in range(2)]

        # ---- phase 1: projections -------------------------------------
        with tc.tile_pool(name="proj_sb", bufs=2) as proj_sb, \
             tc.tile_pool(name="ps_kq", bufs=3, space="PSUM") as ps_kq, \
             tc.tile_pool(name="ps_vt", bufs=2, space="PSUM") as ps_vt:

            # q4 projections (from islice inputs; bq folded via ACT bias)
            for xi in range(2):
                if xi == 0:
                    xi_sb = x1i_sb
                else:
                    xi_sb = [proj_sb.tile([128, IH], BF16, tag="x2i",
                                          name="x2i") for _ in range(2)]
                    for kc in range(2):
                        nc.sync.dma_start(out=xi_sb[kc], in_=dram["x2i"][kc])
                for ib in range(4):
                    sl = bass.ts(ib, 512)
                    qp = ps_kq.tile([128, 512], F32, tag="kq", name="qp")
                    for kc in range(2):
                        nc.tensor.matmul(qp, wq_sb[:, kc, :], xi_sb[kc][:, sl],
                                         start=(kc == 0), stop=(kc == 1))
                    nc.scalar.activation(q4_sb[xi][:, sl], qp, AF.Identity,
                                         bias=bq_sb)

            # k4 and vT projections, x2 first (branch 0 needs vT2)
            for xi, xf_name in [(1, "x2f"), (0, "x1f")]:
                for jh in range(2):
                    xf_t = proj_sb.tile([128, 2, IH], BF16, tag="xf",
                                        name="xf")
                    for kc in range(2):
                        nc.sync.dma_start(
                            out=xf_t[:, kc, :],
                            in_=dram[xf_name][kc][:, jh * IH:(jh + 1) * IH])
                    for jb in range(4):
                        sl = bass.ts(jb, 512)
                        kp = ps_kq.tile([128, 512], F32, tag="kq", name="kp")
                        for kc in range(2):
                            nc.tensor.matmul(kp, wk_sb[:, kc, :],
                                             xf_t[:, kc, sl],
                                             start=(kc == 0), stop=(kc == 1))
                        nc.scalar.activation(k4_sb[xi][jh][:, sl], kp, AF.Copy)
                    for g in range(4):
                        vtp = ps_vt.tile([128, 4, C], F32, tag="vt",
                                         name="vtp")
                        for s in range(4):
                            jsub = g * 4 + s
                            for kc in range(2):
                                nc.tensor.matmul(
                                    vtp[:, s, :],
                                    xf_t[:, kc, bass.ts(jsub, 128)],
                                    wv_sb[:, kc, :],
                                    start=(kc == 0), stop=(kc == 1))
                        nc.vector.tensor_copy(
                            vT_sb[xi][jh][:, bass.ds(g * 4 * C, 4 * C)],
                            vtp.rearrange("p a c -> p (a c)"))
            nc.sync.dma_start(out=bce_sb, in_=dram["bce"])
            for kc in range(2):
                nc.sync.dma_start(out=wcx_sb[:, kc, :], in_=dram["wctx"][kc])
                nc.sync.dma_start(out=wca_sb[:, kc, :], in_=dram["wcta"][kc])

        # r-fold PSUM bank and r-accumulation SBUF outlive 2a (the final
        # block's fold is emitted from inside phase 2b)
        ps_rp = ctx.enter_context(
            tc.tile_pool(name="ps_rp", bufs=1, space="PSUM"))
        racc_pool = ctx.enter_context(tc.tile_pool(name="racc_sb", bufs=1))

        # ---- phase 2a: attention (att_sb <- raw attended; r products) --
        # Software-pipelined emission: the PE queue is strictly in-order,
        # so scores/exp for unit n+1 are emitted BEFORE attended(n) -- the
        # PE runs the next scores while attended(n) waits on exp(n),
        # instead of idling ~1us every group (which also re-throttles HAM).
        with tc.tile_pool(name="attn_sb", bufs=1) as attn_sb, \
             tc.tile_pool(name="ps_att", bufs=1, space="PSUM") as ps_att, \
             tc.tile_pool(name="ps_st", bufs=1, space="PSUM") as ps_st:

            def r_tail(br, ib, racc_v, racc_d):
                """Fold r partials + derive 2b products. Emitted deferred
                (from inside a later block's group loop) so no PE
                instruction ever waits on the accumulate chains."""
                rp = ps_rp.tile([1, 512], F32, tag="rp", bufs=1, name="rp")
                nc.tensor.matmul(rp, ones_f, racc_v, start=True, stop=False)
                nc.tensor.matmul(rp, ones_f, racc_d, start=False, stop=True)
                # 1/r (f32, ~18-bit) straight from PSUM
                nc.vector.reciprocal_approx_fast(rr_sb[br][ib], rp)
                # r line -> bf16 (ACT), then partition-broadcast for the
                # 2b x1-prescale
                nc.scalar.activation(rlb_sb[br][ib], rp, AF.Copy)
                nc.gpsimd.partition_broadcast(rb_sb[br][ib], rlb_sb[br][ib])

            units = [(br, ib, g) for br in range(2) for ib in range(4)
                     for g in range(NG)]
            state = {}   # (br, ib) -> dict with attp/racc/est-per-g

            def emit_scores_exp(br, ib, g):
                q4, k4 = q4_sb[br], k4_sb[br]
                isl = bass.ts(ib, 512)
                st = state.setdefault((br, ib), {"est": {}})
                if g == 0:
                    st["attp"] = [ps_att.tile([128, 512], F32, tag="attp",
                                              bufs=3, name=f"attp{c2}")
                                  for c2 in range(2)]
                    st["racc_v"] = None
                    st["racc_d"] = racc_pool.tile([128, 512], F32R,
                                                  tag="raccd", bufs=2,
                                                  name="racc_d")
                jcs = (2 * g, 2 * g + 1)
                jh = g // (NG // 2)
                jloc = [jc - jh * (NJC // 2) for jc in jcs]
                stp = ps_st.tile([128, 2, 512], F32, tag="stp",
                                 bufs=2, name="stp")
                for t in range(2):
                    nc.tensor.matmul(
                        stp[:, t, :],
                        k4[jh][32 * t:32 * (t + 1), bass.ts(jloc[t], 128)],
                        q4[32 * t:32 * (t + 1), isl],
                        start=True, stop=True, tile_position=(32 * t, 0))
                est = attn_sb.tile([128, 2, 512], BF16, tag="est",
                                   bufs=8, name="est")
                nc.scalar.activation(est.rearrange("p a n -> p (a n)"),
                                     stp.rearrange("p a n -> p (a n)"),
                                     AF.Exp)
                st["est"][g] = est

            pending = None
            emit_scores_exp(0, 0, 0)
            for n, (br, ib, g) in enumerate(units):
                if n + 1 < len(units):
                    emit_scores_exp(*units[n + 1])
                st = state[(br, ib)]
                est, attp = st["est"].pop(g), st["attp"]
                jcs = (2 * g, 2 * g + 1)
                jh = g // (NG // 2)
                jloc = [jc - jh * (NJC // 2) for jc in jcs]
                vT = vT_sb[1 - br]
                for t in range(2):
                    for c2 in range(2):
                        nc.tensor.matmul(
                            attp[c2],
                            vT[jh][:, bass.ds(jloc[t] * C + c2 * 128, 128)],
                            est[:, t, :],
                            start=(g == 0 and t == 0),
                            stop=(g == NG - 1 and t == 1))
                if g == NG - 1:
                    # release attp first: casts jump the DVE queue ahead of
                    # this group's r ops so the next block's attended can
                    # allocate from the 3-deep attp ring without waiting
                    isl = bass.ts(ib, 512)
                    for c2 in range(2):
                        nc.vector.tensor_copy(att_sb[br][c2][:, isl],
                                              attp[c2])
                # r partials: pair-sum the two strips; even groups chain on
                # the DVE (ping-pong), odd groups on one DMA-accumulate
                # chain -- keeps both DVE and gpsimd under the PE group time
                if g % 2 == 0:
                    rtmp = racc_pool.tile([128, 512], BF16, tag="rtmpb",
                                          bufs=2, name="rtmp_b")
                    nc.vector.tensor_tensor(rtmp, est[:, 0, :],
                                            est[:, 1, :], ALU.add)
                    rv = racc_pool.tile([128, 512], F32R, tag="raccv",
                                        bufs=2, name="racc_v")
                    if st["racc_v"] is None:
                        nc.vector.tensor_copy(rv, rtmp)
                    else:
                        nc.vector.tensor_tensor(rv, st["racc_v"], rtmp,
                                                ALU.add)
                    st["racc_v"] = rv
                elif g == 1:
                    nc.vector.tensor_tensor(st["racc_d"], est[:, 0, :],
                                            est[:, 1, :], ALU.add)
                else:
                    rtmp = racc_pool.tile([128, 512], F32R, tag="rtmp",
                                          bufs=3, name="rtmp")
                    nc.vector.tensor_tensor(rtmp, est[:, 0, :],
                                            est[:, 1, :], ALU.add)
                    nc.gpsimd.dma_start(out=st["racc_d"], in_=rtmp,
                                        accum_op=ALU.add)
                if g == 8 and pending is not None:
                    r_tail(*pending)
                    pending = None
                if g == NG - 1:
                    pending = (br, ib, st["racc_v"], st["racc_d"])
                    del state[(br, ib)]
            last_pending = pending

        # ---- phase 2b: combines, back-to-back on the PE ----------------
        # Same one-ahead trick: outp/osb of block n are emitted after the
        # cp matmuls of block n+1 so the PE never waits on the Abs ACT.
        with tc.tile_pool(name="cmb_sb", bufs=1) as cmb_sb, \
             tc.tile_pool(name="ps_cmb", bufs=1, space="PSUM") as ps_cmb:

            def emit_cp(br, ib):
                isl = bass.ts(ib, 512)
                x1r = cmb_sb.tile([128, 2, 512], BF16, tag="x1r",
                                  bufs=2, name="x1r")
                for kc in range(2):
                    nc.vector.tensor_tensor(x1r[:, kc, :],
                                            x1i_sb[kc][:, isl],
                                            rb_sb[br][ib], ALU.mult)
                absb = []
                for c2 in range(2):
                    cp = ps_cmb.tile([128, 512], F32, tag="cp",
                                     bufs=3, name="cp")
                    for kc in range(2):
                        nc.tensor.matmul(cp, wcx_sb[:, kc, bass.ts(c2, 128)],
                                         x1r[:, kc, :],
                                         start=(kc == 0), stop=False)
                    nc.tensor.matmul(cp, bce_sb[:, c2, :], rlb_sb[br][ib],
                                     start=False, stop=False)
                    for kc in range(2):
                        nc.tensor.matmul(cp, wca_sb[:, kc, bass.ts(c2, 128)],
                                         att_sb[br][kc][:, isl],
                                         start=False, stop=(kc == 1))
                    ab = cmb_sb.tile([128, 512], BF16, tag="absb",
                                     bufs=4, name="absb")
                    nc.scalar.activation(ab, cp, AF.Abs)
                    absb.append(ab)
                return absb

            def emit_out(br, ib, absb):
                isl = bass.ts(ib, 512)
                outp = ps_cmb.tile([1, 512], F32, tag="outp", bufs=2,
                                   name="outp")
                for c2 in range(2):
                    nc.tensor.matmul(outp, ones_bf, absb[c2],
                                     start=(c2 == 0), stop=(c2 == 1))
                osb = cmb_sb.tile([1, 512], F32, tag="osb", bufs=2,
                                  name="osb")
                nc.vector.tensor_tensor(osb, outp, rr_sb[br][ib], ALU.mult)
                nc.sync.dma_start(out=out_d[br:br + 1, isl], in_=osb)

            blocks = [(br, ib) for br in range(2) for ib in range(4)]
            prev = None
            for bi, (br, ib) in enumerate(blocks):
                absb = emit_cp(br, ib)
                if prev is not None:
                    emit_out(*prev)
                prev = (br, ib, absb)
                if bi == 2:
                    r_tail(*last_pending)
            emit_out(*prev)


_NC_CACHE = {}


def _get_nc():
    if "nc" not in _NC_CACHE:
        nc = bacc.Bacc("TRN2", debug=False, enable_asserts=False,
                       target_bir_lowering=False, enable_partition_id=False)
        with tile.TileContext(nc) as tc:
            build_program(nc, tc)
        nc.compile()
        _NC_CACHE["nc"] = nc
    return _NC_CACHE["nc"]


def host_inputs(x1, x2, Wq, bq, Wk, bk, Wv, bv, Wc, bc):
    """Build the 8 per-core input maps (host-side sharding/layout only)."""
    f = np.float32
    bf = ml_dtypes.bfloat16
    x1 = np.asarray(x1, f); x2 = np.asarray(x2, f)
    Wq = np.asarray(Wq, f); bq = np.asarray(bq, f)
    Wk = np.asarray(Wk, f)
    Wv = np.asarray(Wv, f); bv = np.asarray(bv, f)
    Wc = np.asarray(Wc, f); bc = np.asarray(bc, f)

    # 4x row-replicated q/k projection weights -> q4/k4 [128, n] layouts
    Wq4 = np.tile(Wq, (4, 1))            # [128, 256]
    Wk4 = np.tile(Wk, (4, 1))
    wqt = np.ascontiguousarray(Wq4.T.reshape(2, 128, 128)).astype(bf)
    wkt = np.ascontiguousarray(Wk4.T.reshape(2, 128, 128)).astype(bf)
    bq4 = np.tile(bq, 4).reshape(128, 1).copy()
    wvt = np.ascontiguousarray(Wv.T.reshape(2, 128, C)).astype(bf)
    WcT = np.ascontiguousarray(Wc.T)     # [512, 256]
    wctx = WcT[:C].reshape(2, 128, C).astype(bf)
    wcta = WcT[C:].reshape(2, 128, C).astype(bf)
    bce = (bc + Wc[:, C:] @ bv).reshape(1, 2, 128).astype(bf)

    in_maps = []
    for core in range(NCORES):
        b, h = divmod(core, 2)
        x1f = x1[b].reshape(C, N).reshape(2, 128, N)
        x2f = x2[b].reshape(C, N).reshape(2, 128, N)
        in_maps.append({
            "x1f": np.ascontiguousarray(x1f).astype(bf),
            "x2f": np.ascontiguousarray(x2f).astype(bf),
            "x1i": np.ascontiguousarray(
                x1f[:, :, h * IH:(h + 1) * IH]).astype(bf),
            "x2i": np.ascontiguousarray(
                x2f[:, :, h * IH:(h + 1) * IH]).astype(bf),
            "wqt": wqt, "wkt": wkt, "wvt": wvt,
            "wctx": wctx, "wcta": wcta,
            "bq": bq4, "bce": bce,
            "onesc": np.ones((128, 1), f),
        })
    return in_maps


def assemble(results):
    """results: list of 8 dicts with 'out' [2, IH] -> (out1, out2) full."""
    outs = []
    for row in range(2):
        full = np.empty((B, 1, HH, WW), np.float32)
        for b in range(B):
            half0 = results[2 * b]["out"][row]
            half1 = results[2 * b + 1]["out"][row]
            full[b, 0] = np.concatenate([half0, half1]).reshape(HH, WW)
        outs.append(full)
    return outs[0], outs[1]


def kernel(x1, x2, Wq, bq, Wk, bk, Wv, bv, Wc, bc):
    in_maps = host_inputs(x1, x2, Wq, bq, Wk, bk, Wv, bv, Wc, bc)
    nc = _get_nc()
    res = run_bass_kernel_spmd(nc, in_maps, core_ids=list(range(NCORES)))
    return assemble(res.results)
